# revision 40
# baseline (speedup 1.0000x reference)
"""Trainium2 Bass kernel for CausalAttentiveStatisticsPooling (v3).

Per batch element b (data-parallel over 8 cores):
  c_mean   = cumsum(x)/count, c_std = sqrt(cumsum(x^2)/count - c_mean^2)
  h        = tanh(w1^T [x; c_mean; c_std] + b1); scores = w2^T h + b2 per key
  attn     = causal softmax -> e_j/E_i with e = exp(scores)*mask, E = cumsum(e)
  w_mean_i = R_i*cumsum(e*x)_i, w_var_i = R_i*cumsum(e*x^2)_i - w_mean_i^2
  out      = [sum_i<L w_mean_i/L, sum_i<L sqrt(w_var_i)/L]

v3 structure (cost-model-driven):
  - All "total"/reduction matmuls are tall-skinny (output free size 1):
    per-chunk totals of x/x^2/(e*x)/(e*x^2) land as (C-part, chunk) columns
    via 128-high lhsT blocks; PE transposes (+identity) turn them into
    (16, C) carry rows for the chunk-carry broadcast-add matmuls.
  - final_mean/final_std are tall-skinny accumulations into a (128,8)
    PSUM tile (4 c-blocks x {mean,std}), transposed once at the end.
  - Squares moved off ACT: sqm = (s1*rcnt2)*s1 on Pool STT; var on DVE STT.
  - ACT only does sqrt/tanh/exp + small copies.
  - No explicit act-table loads (walrus re-inserts them at NEFF lowering).
"""

import numpy as np
import ml_dtypes

B, C, T, A = 8, 512, 2048, 128
NCH = T // 128  # 16 T-chunks
NEG = -30000.0
EPSC = 1e-12

BF = ml_dtypes.bfloat16

# f32 blob columns
CF_RCNT = 0         # (128,16)
CF_RCNT2 = 16       # (128,16)
CF_MASKEXP = 32     # (128,16)
CF_B1 = 48          # (128,1)
CF_EPS = 49         # (128,1)
CF_SUTRI16 = 50     # (16,16) rows 0:16
CF_LTRI16 = 66      # (16,16) rows 0:16
CF_ONESC = 82       # (128,1)
CF_IDENT = 83       # (128,128)
NF = 211
# f32r blob columns
CR_TRIL = 0         # (128,128)
CR_FINALW = 128     # (128,16)
CR_ONESC = 144      # (128,1)
NR = 145
# bf16 blob columns
CB_TRIU = 0         # (128,128)
CB_SUTRI16B = 128   # (16,16) rows 0:16
CB_W2 = 144         # (128,1)
CB_ONESC = 145      # (128,1)
CB_IDENTB = 146     # (128,128)
CB_FWB = 274        # (128,16)
NB = 290

_CACHE = {}


def _build():
    import concourse.bass as bass
    import concourse.mybir as mybir
    import concourse.tile as tile
    from concourse import bacc

    f32 = mybir.dt.float32
    f32r = mybir.dt.float32r
    bf16 = mybir.dt.bfloat16
    f8 = mybir.dt.float8e4
    DR = mybir.MatmulPerfMode.DoubleRow
    AF = mybir.ActivationFunctionType
    OP = mybir.AluOpType

    nc = bacc.Bacc("TRN2", target_bir_lowering=False, debug=False,
                   num_devices=8)

    def din(name, shape, dt):
        return nc.dram_tensor(name, shape, dt, kind="ExternalInput").ap()

    d_xT = din("xT", (T, C), f8)
    d_x2 = din("x2q", (T, C), f8)
    d_c8 = din("cst8", (128, 129), f8)
    d_xN = din("xN", (C, T), f8)
    d_w1x = din("w1x", (128, 8 * A), f8)
    d_w1s = din("w1s", (128, 4 * A), bf16)
    d_cf = din("cstf", (128, NF), f32)
    d_cr = din("cstr", (128, NR), f32r)
    d_cb = din("cstb", (128, NB), bf16)
    d_rb = din("rcntb", (128, T), bf16)
    # (128, 8) layout: col k<4 = final_mean c-block k, col 4+k = final_std;
    # host side untangles to (2C,)
    d_out = nc.dram_tensor("out", (128, 8), f32, kind="ExternalOutput").ap()

    from contextlib import ExitStack
    with tile.TileContext(nc) as tc:
        with ExitStack() as stack:
            def pool(name, bufs, space=None):
                kw = {"space": space} if space else {}
                return stack.enter_context(
                    tc.tile_pool(name=name, bufs=bufs, **kw))
            big = pool("big", 1)
            consts = pool("consts", 1)
            colp = pool("colp", 1)
            sqmp = pool("sqmp", 3)
            v1p = pool("v1p", 4)
            csp = pool("csp", 4)
            natp = pool("natp", 8)
            hp = pool("hp", 4)
            hh = pool("hh", 4)
            wtp = pool("wtp", 6)
            zpp = pool("zpp", 3)
            v2b = pool("v2b", 3)
            rowp = pool("rowp", 1)
            ps_s = pool("ps_s", 2, "PSUM")
            ps_s1 = pool("ps_s1", 2, "PSUM")
            ps_tot = pool("ps_tot", 1, "PSUM")
            ps_ca = pool("ps_ca", 1, "PSUM")
            ps_cb = pool("ps_cb", 1, "PSUM")
            # ---------------- DMAs (x chunks first on both queues) --------
            t_cb = consts.tile([128, NB], bf16)
            nc.sync.dma_start(t_cb, d_cb)
            t_xT = big.tile([128, NCH, C], f8)
            x2 = big.tile([128, NCH, C], f8)
            t_c8 = consts.tile([128, 129], f8)
            nc.sync.dma_start(t_c8, d_c8)
            d_xT_r = d_xT.rearrange("(n p) c -> p n c", p=128)
            d_x2_r = d_x2.rearrange("(n p) c -> p n c", p=128)
            t_xN = big.tile([128, 4, T], f8)
            d_xN_r = d_xN.rearrange("(n p) t -> p n t", p=128)
            t_w1x = consts.tile([128, 8, A], f8)
            t_w1s = consts.tile([128, 4, A], bf16)
            t_cf = consts.tile([128, NF], f32)
            nc.scalar.dma_start(t_cf, d_cf)
            for q in range(8):
                # sync: chunks 0-7 in order; scalar: chunks 8-15
                eng = nc.sync if q < 4 else nc.scalar
                eng.dma_start(t_xT[:, 2 * q:2 * (q + 1), :],
                              d_xT_r[:, 2 * q:2 * (q + 1), :])
            for i in range(NCH):
                eng = nc.vector if i < 8 else nc.gpsimd
                eng.tensor_mul(x2[:, i, :], t_xT[:, i, :],
                               t_xT[:, i, :])
            nc.scalar.dma_start(t_xN[:, :, 0:1024], d_xN_r[:, :, 0:1024])
            nc.scalar.dma_start(t_xN[:, :, 1024:2048], d_xN_r[:, :, 1024:2048])
            nc.scalar.dma_start(t_w1, d_w1.rearrange("p (n a) -> p n a", n=12))
            rbp = consts.tile([128, 4, 512], bf16)
            nc.scalar.dma_start(rbp,
                                d_rb.rearrange("p (g t) -> p g t", g=4))
            t_cr = consts.tile([128, NR], f32r)
            nc.sync.dma_start(t_cr, d_cr)

            # const views
            t_triub = t_cb[:, CB_TRIU:CB_TRIU + 128]
            t_triu8 = t_c8[:, 0:128]
            t_ones8 = t_c8[:, 128:129]
            t_w2b = t_cb[:, CB_W2:CB_W2 + 1]
            t_onescb = t_cb[:, CB_ONESC:CB_ONESC + 1]
            t_identb = t_cb[:, CB_IDENTB:CB_IDENTB + 128]
            t_fwb = t_cb[:, CB_FWB:CB_FWB + 16]
            t_rcnt = t_cf[:, CF_RCNT:CF_RCNT + 16]
            t_rcnt2 = t_cf[:, CF_RCNT2:CF_RCNT2 + 16]
            t_rcnt2 = t_cf[:, CF_RCNT2:CF_RCNT2 + 16]
            t_maskexp = t_cf[:, CF_MASKEXP:CF_MASKEXP + 16]
            t_b1 = t_cf[:, CF_B1:CF_B1 + 1]
            t_sutri16 = t_cf[0:16, CF_SUTRI16:CF_SUTRI16 + 16]
            t_ltri16 = t_cf[0:16, CF_LTRI16:CF_LTRI16 + 16]
            t_onescf = t_cf[:, CF_ONESC:CF_ONESC + 1]
            t_identf = t_cf[:, CF_IDENT:CF_IDENT + 128]
            t_tril = t_cr[:, CR_TRIL:CR_TRIL + 128]
            t_finalw = t_cr[:, CR_FINALW:CR_FINALW + 16]

            def csel_ap(i):
                # strict-lower-than-i selector col, broadcast to (16,128)
                sl = t_cb[0:16, CB_SUTRI16B + i:CB_SUTRI16B + i + 1]
                return bass.AP(tensor=sl.tensor, offset=sl.offset,
                               ap=[[sl.ap[0][0], 16], [0, 128]])

            def bcast16(sb):
                # (16,1) sbuf column -> (16,128) free-broadcast lhsT
                return bass.AP(tensor=sb.tensor, offset=sb.offset,
                               ap=[[sb.ap[0][0], 16], [0, 128]])

            # ------- phase-1 chunk totals (tall-skinny, two halves) --------
            # carry rows for chunks 1..8 only need chunk 0..7 totals, so the
            # first half unblocks the chunk pipeline before xT fully loads.
            totC1 = ps_tot.tile([128, 4, 16], f32, tag="tX")
            totC2 = ps_tot.tile([128, 4, 16], f32, tag="tY")
            totC1b = colp.tile([128, 4, 16], bf16)
            totC2b = colp.tile([128, 4, 16], bf16)
            trowC = rowp.tile([16, 8, 128], bf16)
            nc.vector.memset(trowC[:], 0.0)
            trowC1 = trowC[:, 0:4, :]
            trowC2 = trowC[:, 4:8, :]

            def totC_half(h):
                lo, hi = 8 * h, 8 * h + 8
                for i in range(lo, hi):
                    for k in range(4):
                        nc.tensor.matmul(totC1[:, k, i:i + 1],
                                         t_xT[:, i, 128 * k:128 * (k + 1)],
                                         t_ones8, start=True, stop=True)
                        nc.tensor.matmul(totC2[:, k, i:i + 1],
                                         x2[:, i, 128 * k:128 * (k + 1)],
                                         t_ones8, start=True, stop=True)
                nc.scalar.activation(totC1b[:, :, lo:hi], totC1[:, :, lo:hi],
                                     AF.Copy)
                nc.scalar.activation(totC2b[:, :, lo:hi], totC2[:, :, lo:hi],
                                     AF.Copy)
                trow_ps = ps_cb.tile([8, 8, 128], bf16, tag="cB",
                                     name=f"trow_ps{h}")
                for k in range(4):
                    nc.tensor.matmul(trow_ps[:, k, :], totC1b[:, k, lo:hi],
                                     t_identb, is_transpose=True,
                                     start=True, stop=True)
                    nc.tensor.matmul(trow_ps[:, 4 + k, :],
                                     totC2b[:, k, lo:hi],
                                     t_identb, is_transpose=True,
                                     start=True, stop=True)
                nc.scalar.activation(trowC[lo:hi, :, :], trow_ps[:], AF.Copy)

            # ---------------- phase 1 + MLP ----------------
            nats = [None] * NCH

            def p1_chunk(i):
                s1 = ps_s.tile([128, C], f32, tag="sA", name=f"s1_{i}")
                nc.tensor.matmul(s1[:], t_triu8, t_xT[:, i, :],
                                 start=True, stop=(i == 0))
                s2 = ps_s1.tile([128, C], f32, tag="sB", name=f"s2_{i}")
                nc.tensor.matmul(s2[:], t_triu8, x2[:, i, :],
                                 start=True, stop=(i == 0))
                if i > 0:
                    nc.tensor.matmul(s1[:], csel_ap(i), trowC1,
                                     start=False, stop=True)
                    nc.tensor.matmul(s2[:], csel_ap(i), trowC2,
                                     start=False, stop=True)
                sqm = sqmp.tile([128, C], bf16, tag="sqm", name=f"sqm_{i}")
                if i % 2 == 0:
                    # gpsimd cannot read PSUM; split squares DVE/ACT instead
                    nc.vector.scalar_tensor_tensor(sqm[:], s1[:],
                                                   t_rcnt2[:, i:i + 1], s1[:],
                                                   op0=OP.mult, op1=OP.mult)
                else:
                    nc.scalar.activation(sqm[:], s1[:], AF.Square,
                                         scale=t_rcnt[:, i:i + 1])
                var1 = v1p.tile([128, C], bf16, tag="v1", name=f"var1_{i}")
                nc.vector.scalar_tensor_tensor(var1[:], s2[:],
                                               t_rcnt[:, i:i + 1], sqm[:],
                                               op0=OP.mult, op1=OP.subtract)
                if i == 0:
                    # only chunk 0 can see true variance near 0 (count<128)
                    nc.gpsimd.tensor_scalar_max(var1[:], var1[:], EPSC)
                cs = csp.tile([128, C], bf16, tag="cs", name=f"cs_{i}")
                nc.scalar.activation(cs[:], var1[:], AF.Sqrt)
                nat = natp.tile([128, 4, 128], bf16, tag="nat",
                                name=f"nat_{i}")
                nc.sync.dma_start_transpose(nat[:], cs[:])
                nats[i] = nat

            p1_chunk(0)
            totC_half(0)
            for i in range(1, 8):
                p1_chunk(i)
            totC_half(1)
            for i in range(8, NCH):
                p1_chunk(i)

            # ---------------- MLP (fills the tanh gap) ----------------
            Pm_sb = big.tile([128, T], f32r)
            zeros512 = consts.tile([128, 512], f32)
            nc.vector.memset(zeros512[:], 0.0)
            hpres = [None] * 4
            for g in range(4):
                pm_ps = ps_ca.tile([A, 512], f32, tag="cA")
                for p2 in range(2):
                    nc.tensor.matmul(
                        pm_ps[:], t_w1x[:, 4 + 2 * p2:6 + 2 * p2, :],
                        t_xN[:, 2 * p2:2 * p2 + 2, 512 * g:512 * (g + 1)],
                        start=(p2 == 0), stop=(p2 == 1), perf_mode=DR)
                sl = Pm_sb[:, 512 * g:512 * (g + 1)]
                init = (0.0 if g == 0
                        else Pm_sb[:, 512 * g - 1:512 * g].bitcast(f32))
                nc.vector.tensor_tensor_scan(sl, pm_ps[:], zeros512[:],
                                             initial=init,
                                             op0=OP.add, op1=OP.add)
                nc.gpsimd.tensor_mul(sl, sl.bitcast(f32), rbp[:, g, :])
                ph = ps_tot.tile([A, 512], f32, tag=("tX" if g % 2 == 0
                                                     else "tY"))
                for p2 in range(2):
                    nc.tensor.matmul(
                        ph[:], t_w1x[:, 2 * p2:2 * p2 + 2, :],
                        t_xN[:, 2 * p2:2 * p2 + 2, 512 * g:512 * (g + 1)],
                        start=(p2 == 0), stop=False, perf_mode=DR)
                for k in range(4):
                    for cb in range(4):
                        nc.tensor.matmul(
                            ph[:, 128 * k:128 * (k + 1)],
                            t_w1s[:, cb, :],
                            nats[4 * g + k][:, cb, :],
                            start=False, stop=(cb == 3))
                hpre = hp.tile([A, 512], f32, tag="hpre")
                nc.vector.scalar_tensor_tensor(
                    hpre[:], ph[:], t_b1,
                    Pm_sb[:, 512 * g:512 * (g + 1)].bitcast(f32),
                    op0=OP.add, op1=OP.add)
                hpres[g] = hpre

            # ---- fused attention loop: per group g, the loop-A
            # machinery (tanh/scores/exp/E/R) feeds straight into phase-2
            # chunk work; carry rows for group g land just in time for its
            # own chunks' corrections (chunk 4g needs only prior groups).
            eEp = ps_cb.tile([128, NCH], f32, tag="cB")
            eTf = colp.tile([128, NCH], f32)
            eTb = colp.tile([128, NCH], bf16)
            nc.vector.memset(eTb[:], 0.0)
            etot = colp.tile([16, 1], f32)
            R_col = colp.tile([128, NCH], f32)
            R2_col = colp.tile([128, NCH], f32)
            eR = colp.tile([128, NCH], f32)
            e8 = colp.tile([128, NCH], f8)
            wsall = big.tile([128, NCH, C], bf16)
            trowE = rowp.tile([16, 8, 128], bf16)
            nc.vector.memset(trowE[:], 0.0)
            trowE1 = trowE[:, 0:4, :]
            trowE2 = trowE[:, 4:8, :]
            # tX/tY slots are free once the MLP's ph tiles are consumed
            outaccM = ps_tot.tile([128, 4], f32, tag="tX")
            outaccS = ps_tot.tile([128, 4], f32, tag="tY")

            def p2_mains(ii):
                wtri = wtp.tile([128, 128], bf16, tag="wtri",
                                name=f"wtri_{ii}")
                nc.gpsimd.tensor_scalar_mul(
                    wtri[:], t_triub, eR[:, ii:ii + 1])
                mp = ps_s.tile([128, C], f32, tag="sA", name=f"mp_{ii}")
                nc.tensor.matmul(mp[:], wtri[:], t_xT[:, ii, :],
                                 start=True, stop=(ii == 0))
                ap = ps_s1.tile([128, C], f32, tag="sB", name=f"ap_{ii}")
                nc.tensor.matmul(ap[:], wtri[:], x2[:, ii, :],
                                 start=True, stop=(ii == 0))
                return mp, ap

            def p2_csel(ii, mp, ap):
                if ii > 0:
                    nc.tensor.matmul(mp[:], csel_ap(ii), trowE1,
                                     start=False, stop=True)
                    nc.tensor.matmul(ap[:], csel_ap(ii), trowE2,
                                     start=False, stop=True)

            def p2_evict(ii, mp, ap):
                zp = zpp.tile([128, C], bf16, tag="zp", name=f"zp_{ii}")
                if ii % 2 == 0:
                    nc.scalar.activation(zp[:], mp[:], AF.Square,
                                         scale=R_col[:, ii:ii + 1])
                else:
                    nc.vector.scalar_tensor_tensor(zp[:], mp[:],
                                                   R2_col[:, ii:ii + 1],
                                                   mp[:],
                                                   op0=OP.mult, op1=OP.mult)
                var2 = v2b.tile([128, C], bf16, tag="v2", name=f"var2_{ii}")
                nc.vector.scalar_tensor_tensor(var2[:], ap[:],
                                               R_col[:, ii:ii + 1], zp[:],
                                               op0=OP.mult, op1=OP.subtract)
                nc.gpsimd.tensor_scalar_max(var2[:], var2[:], EPSC)
                nc.scalar.activation(wsall[:, ii, :], var2[:], AF.Sqrt)

            for g in range(4):
                i0 = 4 * g
                h = hh.tile([A, 512], bf16, tag="h_sb", name=f"h_{g}")
                nc.scalar.activation(h[:], hpres[g][:], AF.Tanh)
                ps_sc = ps_ca.tile([128, 4], f32, tag="cA",
                                   name=f"ps_sc_{g}")
                for k in range(4):
                    nc.tensor.matmul(
                        ps_sc[:, k:k + 1],
                        h[:, 128 * k:128 * (k + 1)],
                        t_w2b, start=True, stop=True)
                nc.vector.tensor_add(eTf[:, i0:i0 + 4],
                                     ps_sc[:],
                                     t_maskexp[:, i0:i0 + 4])
                nc.scalar.activation(
                    eTb[:, i0:i0 + 4], eTf[:, i0:i0 + 4], AF.Exp)
                ps_et = ps_ca.tile([16, 1], f32, tag="cA",
                                   name=f"ps_et_{g}")
                nc.tensor.matmul(ps_et[:], eTb[:], t_onescb,
                                 start=True, stop=True)
                nc.vector.tensor_copy(etot[:], ps_et[:])
                nc.tensor.matmul(eEp[:, i0:i0 + 4], t_triub,
                                 eTb[:, i0:i0 + 4],
                                 start=True, stop=False)
                nc.tensor.matmul(eEp[:, i0:i0 + 4], bcast16(etot),
                                 t_sutri16[:, i0:i0 + 4],
                                 start=False, stop=True)
                nc.vector.reciprocal(R_col[:, i0:i0 + 4],
                                     eEp[:, i0:i0 + 4])
                nc.vector.tensor_mul(R2_col[:, i0:i0 + 4],
                                     R_col[:, i0:i0 + 4],
                                     R_col[:, i0:i0 + 4])
                nc.vector.tensor_copy(eR[:, i0:i0 + 4], eTb[:, i0:i0 + 4])

                # chunk 4g: its carries come from prior groups only
                mp0, ap0 = p2_mains(i0)
                p2_csel(i0, mp0, ap0)
                # group-g e-weighted totals (both quantities in one tile)
                totE_g = ps_ca.tile([128, 8, 4], f32, tag="cA",
                                    name=f"totE_{g}")
                for kk in range(4):
                    ii = i0 + kk
                    for k in range(4):
                        nc.tensor.matmul(totE_g[:, k, kk:kk + 1],
                                         t_xT[:, ii, 128 * k:128 * (k + 1)],
                                         eTb[:, ii:ii + 1],
                                         start=True, stop=True)
                        nc.tensor.matmul(totE_g[:, 4 + k, kk:kk + 1],
                                         x2[:, ii, 128 * k:128 * (k + 1)],
                                         eTb[:, ii:ii + 1],
                                         start=True, stop=True)
                totEb_g = colp.tile([128, 8, 4], bf16, name=f"totEb_{g}",
                                    tag="totEb", bufs=2)
                nc.scalar.activation(totEb_g[:], totE_g[:], AF.Copy)
                mp1, ap1 = p2_mains(i0 + 1)
                trow_ps = ps_ca.tile([4, 8, 128], bf16, tag="cA",
                                     name=f"trow_ps_{g}")
                for k in range(8):
                    nc.tensor.matmul(trow_ps[:, k, :], totEb_g[:, k, :],
                                     t_identb, is_transpose=True,
                                     start=True, stop=True)
                nc.scalar.activation(trowE[i0:i0 + 4, :, :],
                                     trow_ps[:], AF.Copy)
                p2_evict(i0, mp0, ap0)
                p2_csel(i0 + 1, mp1, ap1)
                p2_evict(i0 + 1, mp1, ap1)
                for kk in (2, 3):
                    ii = i0 + kk
                    mp, ap = p2_mains(ii)
                    p2_csel(ii, mp, ap)
                    p2_evict(ii, mp, ap)
                if g == 3:
                    # G machinery + final-mean as soon as R is complete
                    RLr = colp.tile([128, NCH], f32r)
                    nc.vector.tensor_mul(RLr[:], R_col[:],
                                         t_finalw.bitcast(f32))
                    ps_G = ps_cb.tile([128, NCH], f32, tag="cB")
                    nc.tensor.matmul(ps_G[:], t_tril, RLr[:],
                                     start=True, stop=False)
                    ps_rt = ps_ca.tile([16, 1], f32, tag="cA")
                    nc.tensor.matmul(ps_rt[:], RLr[:].bitcast(f32),
                                     t_onescf, start=True, stop=True)
                    rtot = colp.tile([16, 1], f32)
                    nc.vector.tensor_copy(rtot[:], ps_rt[:])
                    nc.tensor.matmul(ps_G[:], bcast16(rtot), t_ltri16,
                                     start=False, stop=True)
                    wG = colp.tile([128, NCH], f8)
                    nc.vector.tensor_mul(wG[:], eTb[:], ps_G[:])
                    for k in range(4):
                        for ii in range(NCH):
                            nc.tensor.matmul(
                                outaccM[:, k:k + 1],
                                t_xT[:, ii, 128 * k:128 * (k + 1)],
                                wG[:, ii:ii + 1],
                                start=(ii == 0), stop=(ii == NCH - 1))
                    outM = colp.tile([128, 4], f32)
                    nc.scalar.activation(outM[:], outaccM[:], AF.Copy)
                    nc.sync.dma_start(d_out[:, 0:4], outM[:])

            # final-std reductions: one PSUM column at a time
            for k in range(4):
                for ii in range(NCH):
                    nc.tensor.matmul(outaccS[:, k:k + 1],
                                     wsall[:, ii, 128 * k:128 * (k + 1)],
                                     t_fwb[:, ii:ii + 1],
                                     start=(ii == 0),
                                     stop=(ii == NCH - 1))
            outS = colp.tile([128, 4], f32)
            nc.scalar.activation(outS[:], outaccS[:], AF.Copy)
            nc.sync.dma_start(d_out[:, 4:8], outS[:])

    nc.insert_act_table_loads = lambda: None
    nc.compile()
    return nc


def _host_inputs(x, lengths, w1, b1, w2, b2):
    x = np.asarray(x, np.float32)
    lengths = np.asarray(lengths)
    w1 = np.asarray(w1, np.float32)
    b1 = np.asarray(b1, np.float32)
    w2 = np.asarray(w2, np.float32)
    b2 = np.asarray(b2, np.float32)

    sutri16 = np.triu(np.ones((16, 16), np.float32), 1)

    cstf0 = np.zeros((128, NF), np.float32)
    cstf0[:, CF_B1] = b1
    cstf0[:, CF_EPS] = EPSC
    cstf0[0:16, CF_SUTRI16:CF_SUTRI16 + 16] = sutri16
    cstf0[0:16, CF_LTRI16:CF_LTRI16 + 16] = np.tril(
        np.ones((16, 16), np.float32), -1)
    cstf0[:, CF_ONESC] = 1.0
    cstf0[:, CF_IDENT:CF_IDENT + 128] = np.eye(128, dtype=np.float32)

    cstr = np.zeros((128, NR), np.float32)
    cstr[:, CR_TRIL:CR_TRIL + 128] = np.tril(np.ones((128, 128), np.float32))
    cstr[:, CR_ONESC] = 1.0

    cstb0 = np.zeros((128, NB), np.float32)
    cstb0[:, CB_TRIU:CB_TRIU + 128] = np.triu(np.ones((128, 128), np.float32))
    cstb0[0:16, CB_SUTRI16B:CB_SUTRI16B + 16] = sutri16
    cstb0[:, CB_W2] = w2[:, 0]
    cstb0[:, CB_ONESC] = 1.0
    cstb0[:, CB_IDENTB:CB_IDENTB + 128] = np.eye(128, dtype=np.float32)

    tt = np.arange(T)
    F8 = ml_dtypes.float8_e4m3
    cst8 = np.zeros((128, 129), np.float32)
    cst8[:, 0:128] = np.triu(np.ones((128, 128), np.float32))
    cst8[:, 128] = 1.0
    cst8 = cst8.astype(F8)
    w1r = np.ascontiguousarray(
        w1.reshape(12, 128, A).transpose(1, 0, 2).reshape(128, 12 * A))
    w1x8 = w1r[:, :8 * A].astype(F8)
    w1sb = w1r[:, 8 * A:].astype(BF)

    maps = []
    for b in range(B):
        L = int(lengths[b])
        xq8 = np.ascontiguousarray(x[b].T).astype(F8)
        xqf = xq8.astype(np.float32)
        x2q8 = (xqf * xqf).astype(F8)
        rcnt = (1.0 / np.minimum(tt + 1, max(L, 1))).astype(np.float32)
        maskexp = (float(b2[0]) +
                   np.where(tt < L, 0.0, NEG)).astype(np.float32)
        finalw = np.where(tt < L, 1.0 / max(L, 1), 0.0).astype(np.float32)
        cstf = cstf0.copy()
        cstf[:, CF_RCNT:CF_RCNT + 16] = rcnt.reshape(NCH, 128).T
        cstf[:, CF_RCNT2:CF_RCNT2 + 16] = (rcnt * rcnt).reshape(NCH, 128).T
        cstf[:, CF_RCNT2:CF_RCNT2 + 16] = (rcnt * rcnt).reshape(NCH, 128).T
        cstf[:, CF_MASKEXP:CF_MASKEXP + 16] = maskexp.reshape(NCH, 128).T
        cstr_b = cstr.copy()
        cstr_b[:, CR_FINALW:CR_FINALW + 16] = finalw.reshape(NCH, 128).T
        cstb_b = cstb.copy()
        cstb_b[:, CB_FWB:CB_FWB + 16] = finalw.reshape(NCH, 128).T
        cstb = cstb0.copy()
        cstb[:, CB_FWB:CB_FWB + 16] = finalw.reshape(NCH, 128).T
        maps.append({
            "xT": xq8,
            "x2q": x2q8,
            "cst8": cst8,
            "xN": np.ascontiguousarray(x[b]).astype(F8),
            "w1x": w1x8,
            "w1s": w1sb,
            "cstf": cstf,
            "cstr": cstr_b,
            "cstb": cstb.astype(BF),
            "rcntb": np.ascontiguousarray(
                np.broadcast_to(rcnt[None, :], (128, T))).astype(BF),
        })
    return maps


def kernel(x, lengths, w1, b1, w2, b2):
    from concourse.bass_utils import run_bass_kernel_spmd

    if "nc" not in _CACHE:
        _CACHE["nc"] = _build()
    nc = _CACHE["nc"]
    maps = _host_inputs(x, lengths, w1, b1, w2, b2)
    res = run_bass_kernel_spmd(nc, maps, list(range(B))).results
    rows = []
    for b in range(B):
        arr = np.asarray(res[b]["out"], np.float32)  # (128, 8)
        rows.append(np.concatenate([arr[:, 0:4].T.reshape(-1),
                                    arr[:, 4:8].T.reshape(-1)]))
    return np.stack(rows, axis=0).astype(np.float32)


# revision 41
# speedup vs baseline: 1.2541x; 1.2541x over previous
"""Trainium2 Bass kernel for CausalAttentiveStatisticsPooling (v3).

Per batch element b (data-parallel over 8 cores):
  c_mean   = cumsum(x)/count, c_std = sqrt(cumsum(x^2)/count - c_mean^2)
  h        = tanh(w1^T [x; c_mean; c_std] + b1); scores = w2^T h + b2 per key
  attn     = causal softmax -> e_j/E_i with e = exp(scores)*mask, E = cumsum(e)
  w_mean_i = R_i*cumsum(e*x)_i, w_var_i = R_i*cumsum(e*x^2)_i - w_mean_i^2
  out      = [sum_i<L w_mean_i/L, sum_i<L sqrt(w_var_i)/L]

v3 structure (cost-model-driven):
  - All "total"/reduction matmuls are tall-skinny (output free size 1):
    per-chunk totals of x/x^2/(e*x)/(e*x^2) land as (C-part, chunk) columns
    via 128-high lhsT blocks; PE transposes (+identity) turn them into
    (16, C) carry rows for the chunk-carry broadcast-add matmuls.
  - final_mean/final_std are tall-skinny accumulations into a (128,8)
    PSUM tile (4 c-blocks x {mean,std}), transposed once at the end.
  - Squares moved off ACT: sqm = (s1*rcnt2)*s1 on Pool STT; var on DVE STT.
  - ACT only does sqrt/tanh/exp + small copies.
  - No explicit act-table loads (walrus re-inserts them at NEFF lowering).
"""

import numpy as np
import ml_dtypes

B, C, T, A = 8, 512, 2048, 128
NCH = T // 128  # 16 T-chunks
NEG = -30000.0
EPSC = 1e-12

BF = ml_dtypes.bfloat16

# f32 blob columns
CF_RCNT = 0         # (128,16)
CF_RCNT2 = 16       # (128,16)
CF_MASKEXP = 32     # (128,16)
CF_B1 = 48          # (128,1)
CF_EPS = 49         # (128,1)
CF_SUTRI16 = 50     # (16,16) rows 0:16
CF_LTRI16 = 66      # (16,16) rows 0:16
CF_ONESC = 82       # (128,1)
CF_IDENT = 83       # (128,128)
NF = 211
# f32r blob columns
CR_TRIL = 0         # (128,128)
CR_FINALW = 128     # (128,16)
CR_ONESC = 144      # (128,1)
NR = 145
# bf16 blob columns
CB_TRIU = 0         # (128,128)
CB_SUTRI16B = 128   # (16,16) rows 0:16
CB_W2 = 144         # (128,1)
CB_ONESC = 145      # (128,1)
CB_IDENTB = 146     # (128,128)
CB_FWB = 274        # (128,16)
NB = 290

_CACHE = {}


def _build():
    import concourse.bass as bass
    import concourse.mybir as mybir
    import concourse.tile as tile
    from concourse import bacc

    f32 = mybir.dt.float32
    f32r = mybir.dt.float32r
    bf16 = mybir.dt.bfloat16
    f8 = mybir.dt.float8e4
    DR = mybir.MatmulPerfMode.DoubleRow
    AF = mybir.ActivationFunctionType
    OP = mybir.AluOpType

    nc = bacc.Bacc("TRN2", target_bir_lowering=False, debug=False,
                   num_devices=8)

    def din(name, shape, dt):
        return nc.dram_tensor(name, shape, dt, kind="ExternalInput").ap()

    d_xT = din("xT", (T, C), bf16)
    d_xN = din("xN", (C, T), f8)
    d_w1x = din("w1x", (128, 8 * A), f8)
    d_w1s = din("w1s", (128, 4 * A), bf16)
    d_cf = din("cstf", (128, NF), f32)
    d_cr = din("cstr", (128, NR), f32r)
    d_cb = din("cstb", (128, NB), bf16)
    d_rb = din("rcntb", (128, T), bf16)
    # (128, 8) layout: col k<4 = final_mean c-block k, col 4+k = final_std;
    # host side untangles to (2C,)
    d_out = nc.dram_tensor("out", (128, 8), f32, kind="ExternalOutput").ap()

    from contextlib import ExitStack
    with tile.TileContext(nc) as tc:
        with ExitStack() as stack:
            def pool(name, bufs, space=None):
                kw = {"space": space} if space else {}
                return stack.enter_context(
                    tc.tile_pool(name=name, bufs=bufs, **kw))
            big = pool("big", 1)
            consts = pool("consts", 1)
            colp = pool("colp", 1)
            sqmp = pool("sqmp", 3)
            v1p = pool("v1p", 4)
            csp = pool("csp", 4)
            natp = pool("natp", 8)
            hp = pool("hp", 4)
            hh = pool("hh", 4)
            wtp = pool("wtp", 6)
            zpp = pool("zpp", 3)
            v2b = pool("v2b", 3)
            rowp = pool("rowp", 1)
            ps_s = pool("ps_s", 2, "PSUM")
            ps_s1 = pool("ps_s1", 2, "PSUM")
            ps_tot = pool("ps_tot", 1, "PSUM")
            ps_ca = pool("ps_ca", 1, "PSUM")
            ps_cb = pool("ps_cb", 1, "PSUM")
            # ---------------- DMAs (x chunks first on both queues) --------
            t_cb = consts.tile([128, NB], bf16)
            nc.sync.dma_start(t_cb, d_cb)
            t_xT = big.tile([128, NCH, C], bf16)
            x2 = big.tile([128, NCH, C], bf16)
            d_xT_r = d_xT.rearrange("(n p) c -> p n c", p=128)
            t_xN = big.tile([128, 4, T], f8)
            d_xN_r = d_xN.rearrange("(n p) t -> p n t", p=128)
            t_w1x = consts.tile([128, 8, A], f8)
            t_w1s = consts.tile([128, 4, A], bf16)
            t_cf = consts.tile([128, NF], f32)
            nc.scalar.dma_start(t_cf, d_cf)
            for q in range(8):
                # sync: chunks 0-7 in order; scalar: chunks 8-15
                eng = nc.sync if q < 4 else nc.scalar
                eng.dma_start(t_xT[:, 2 * q:2 * (q + 1), :],
                              d_xT_r[:, 2 * q:2 * (q + 1), :])
            for i in range(NCH):
                eng = nc.vector if i < 8 else nc.gpsimd
                eng.tensor_mul(x2[:, i, :], t_xT[:, i, :],
                               t_xT[:, i, :])
            nc.scalar.dma_start(t_xN[:, :, 0:1024], d_xN_r[:, :, 0:1024])
            nc.scalar.dma_start(t_xN[:, :, 1024:2048], d_xN_r[:, :, 1024:2048])
            nc.scalar.dma_start(t_w1, d_w1.rearrange("p (n a) -> p n a", n=12))
            rbp = consts.tile([128, 4, 512], bf16)
            nc.scalar.dma_start(rbp,
                                d_rb.rearrange("p (g t) -> p g t", g=4))
            t_cr = consts.tile([128, NR], f32r)
            nc.sync.dma_start(t_cr, d_cr)

            # const views
            t_triub = t_cb[:, CB_TRIU:CB_TRIU + 128]
            t_w2b = t_cb[:, CB_W2:CB_W2 + 1]
            t_onescb = t_cb[:, CB_ONESC:CB_ONESC + 1]
            t_identb = t_cb[:, CB_IDENTB:CB_IDENTB + 128]
            t_fwb = t_cb[:, CB_FWB:CB_FWB + 16]
            t_rcnt = t_cf[:, CF_RCNT:CF_RCNT + 16]
            t_rcnt2 = t_cf[:, CF_RCNT2:CF_RCNT2 + 16]
            t_rcnt2 = t_cf[:, CF_RCNT2:CF_RCNT2 + 16]
            t_maskexp = t_cf[:, CF_MASKEXP:CF_MASKEXP + 16]
            t_b1 = t_cf[:, CF_B1:CF_B1 + 1]
            t_sutri16 = t_cf[0:16, CF_SUTRI16:CF_SUTRI16 + 16]
            t_ltri16 = t_cf[0:16, CF_LTRI16:CF_LTRI16 + 16]
            t_onescf = t_cf[:, CF_ONESC:CF_ONESC + 1]
            t_identf = t_cf[:, CF_IDENT:CF_IDENT + 128]
            t_tril = t_cr[:, CR_TRIL:CR_TRIL + 128]
            t_finalw = t_cr[:, CR_FINALW:CR_FINALW + 16]

            def csel_ap(i):
                # strict-lower-than-i selector col, broadcast to (16,128)
                sl = t_cb[0:16, CB_SUTRI16B + i:CB_SUTRI16B + i + 1]
                return bass.AP(tensor=sl.tensor, offset=sl.offset,
                               ap=[[sl.ap[0][0], 16], [0, 128]])

            def bcast16(sb):
                # (16,1) sbuf column -> (16,128) free-broadcast lhsT
                return bass.AP(tensor=sb.tensor, offset=sb.offset,
                               ap=[[sb.ap[0][0], 16], [0, 128]])

            # ------- phase-1 chunk totals (tall-skinny, two halves) --------
            # carry rows for chunks 1..8 only need chunk 0..7 totals, so the
            # first half unblocks the chunk pipeline before xT fully loads.
            totC1 = ps_tot.tile([128, 4, 16], f32, tag="tX")
            totC2 = ps_tot.tile([128, 4, 16], f32, tag="tY")
            totC1b = colp.tile([128, 4, 16], bf16)
            totC2b = colp.tile([128, 4, 16], bf16)
            trowC = rowp.tile([16, 8, 128], bf16)
            nc.vector.memset(trowC[:], 0.0)
            trowC1 = trowC[:, 0:4, :]
            trowC2 = trowC[:, 4:8, :]

            def totC_half(h):
                lo, hi = 8 * h, 8 * h + 8
                for i in range(lo, hi):
                    for k in range(4):
                        nc.tensor.matmul(totC1[:, k, i:i + 1],
                                         t_xT[:, i, 128 * k:128 * (k + 1)],
                                         t_onescb, start=True, stop=True)
                        nc.tensor.matmul(totC2[:, k, i:i + 1],
                                         x2[:, i, 128 * k:128 * (k + 1)],
                                         t_onescb, start=True, stop=True)
                nc.scalar.activation(totC1b[:, :, lo:hi], totC1[:, :, lo:hi],
                                     AF.Copy)
                nc.scalar.activation(totC2b[:, :, lo:hi], totC2[:, :, lo:hi],
                                     AF.Copy)
                trow_ps = ps_cb.tile([8, 8, 128], bf16, tag="cB",
                                     name=f"trow_ps{h}")
                for k in range(4):
                    nc.tensor.matmul(trow_ps[:, k, :], totC1b[:, k, lo:hi],
                                     t_identb, is_transpose=True,
                                     start=True, stop=True)
                    nc.tensor.matmul(trow_ps[:, 4 + k, :],
                                     totC2b[:, k, lo:hi],
                                     t_identb, is_transpose=True,
                                     start=True, stop=True)
                nc.scalar.activation(trowC[lo:hi, :, :], trow_ps[:], AF.Copy)

            # ---------------- phase 1 + MLP ----------------
            nats = [None] * NCH

            def p1_chunk(i):
                s1 = ps_s.tile([128, C], f32, tag="sA", name=f"s1_{i}")
                nc.tensor.matmul(s1[:], t_triub, t_xT[:, i, :],
                                 start=True, stop=(i == 0))
                s2 = ps_s1.tile([128, C], f32, tag="sB", name=f"s2_{i}")
                nc.tensor.matmul(s2[:], t_triub, x2[:, i, :],
                                 start=True, stop=(i == 0))
                if i > 0:
                    nc.tensor.matmul(s1[:], csel_ap(i), trowC1,
                                     start=False, stop=True)
                    nc.tensor.matmul(s2[:], csel_ap(i), trowC2,
                                     start=False, stop=True)
                sqm = sqmp.tile([128, C], bf16, tag="sqm", name=f"sqm_{i}")
                if i % 2 == 0:
                    # gpsimd cannot read PSUM; split squares DVE/ACT instead
                    nc.vector.scalar_tensor_tensor(sqm[:], s1[:],
                                                   t_rcnt2[:, i:i + 1], s1[:],
                                                   op0=OP.mult, op1=OP.mult)
                else:
                    nc.scalar.activation(sqm[:], s1[:], AF.Square,
                                         scale=t_rcnt[:, i:i + 1])
                var1 = v1p.tile([128, C], bf16, tag="v1", name=f"var1_{i}")
                nc.vector.scalar_tensor_tensor(var1[:], s2[:],
                                               t_rcnt[:, i:i + 1], sqm[:],
                                               op0=OP.mult, op1=OP.subtract)
                if i == 0:
                    # only chunk 0 can see true variance near 0 (count<128)
                    nc.gpsimd.tensor_scalar_max(var1[:], var1[:], EPSC)
                cs = csp.tile([128, C], bf16, tag="cs", name=f"cs_{i}")
                nc.scalar.activation(cs[:], var1[:], AF.Sqrt)
                nat = natp.tile([128, 4, 128], bf16, tag="nat",
                                name=f"nat_{i}")
                nc.sync.dma_start_transpose(nat[:], cs[:])
                nats[i] = nat

            p1_chunk(0)
            totC_half(0)
            for i in range(1, 8):
                p1_chunk(i)
            totC_half(1)
            for i in range(8, NCH):
                p1_chunk(i)

            # ---------------- MLP (fills the tanh gap) ----------------
            Pm_sb = big.tile([128, T], f32r)
            zeros512 = consts.tile([128, 512], f32)
            nc.vector.memset(zeros512[:], 0.0)
            hpres = [None] * 4
            for g in range(4):
                pm_ps = ps_ca.tile([A, 512], f32, tag="cA")
                for p2 in range(2):
                    nc.tensor.matmul(
                        pm_ps[:], t_w1x[:, 4 + 2 * p2:6 + 2 * p2, :],
                        t_xN[:, 2 * p2:2 * p2 + 2, 512 * g:512 * (g + 1)],
                        start=(p2 == 0), stop=(p2 == 1), perf_mode=DR)
                sl = Pm_sb[:, 512 * g:512 * (g + 1)]
                init = (0.0 if g == 0
                        else Pm_sb[:, 512 * g - 1:512 * g].bitcast(f32))
                nc.vector.tensor_tensor_scan(sl, pm_ps[:], zeros512[:],
                                             initial=init,
                                             op0=OP.add, op1=OP.add)
                nc.gpsimd.tensor_mul(sl, sl.bitcast(f32), rbp[:, g, :])
                ph = ps_tot.tile([A, 512], f32, tag=("tX" if g % 2 == 0
                                                     else "tY"))
                for p2 in range(2):
                    nc.tensor.matmul(
                        ph[:], t_w1x[:, 2 * p2:2 * p2 + 2, :],
                        t_xN[:, 2 * p2:2 * p2 + 2, 512 * g:512 * (g + 1)],
                        start=(p2 == 0), stop=False, perf_mode=DR)
                for k in range(4):
                    for cb in range(4):
                        nc.tensor.matmul(
                            ph[:, 128 * k:128 * (k + 1)],
                            t_w1s[:, cb, :],
                            nats[4 * g + k][:, cb, :],
                            start=False, stop=(cb == 3))
                hpre = hp.tile([A, 512], f32, tag="hpre")
                nc.vector.scalar_tensor_tensor(
                    hpre[:], ph[:], t_b1,
                    Pm_sb[:, 512 * g:512 * (g + 1)].bitcast(f32),
                    op0=OP.add, op1=OP.add)
                hpres[g] = hpre

            # ---- fused attention loop: per group g, the loop-A
            # machinery (tanh/scores/exp/E/R) feeds straight into phase-2
            # chunk work; carry rows for group g land just in time for its
            # own chunks' corrections (chunk 4g needs only prior groups).
            eEp = ps_cb.tile([128, NCH], f32, tag="cB")
            eTf = colp.tile([128, NCH], f32)
            eTb = colp.tile([128, NCH], bf16)
            nc.vector.memset(eTb[:], 0.0)
            etot = colp.tile([16, 1], f32)
            R_col = colp.tile([128, NCH], f32)
            R2_col = colp.tile([128, NCH], f32)
            eR = colp.tile([128, NCH], f32)
            wsall = big.tile([128, NCH, C], bf16)
            trowE = rowp.tile([16, 8, 128], bf16)
            nc.vector.memset(trowE[:], 0.0)
            trowE1 = trowE[:, 0:4, :]
            trowE2 = trowE[:, 4:8, :]
            # tX/tY slots are free once the MLP's ph tiles are consumed
            outaccM = ps_tot.tile([128, 4], f32, tag="tX")
            outaccS = ps_tot.tile([128, 4], f32, tag="tY")

            def p2_mains(ii):
                wtri = wtp.tile([128, 128], bf16, tag="wtri",
                                name=f"wtri_{ii}")
                nc.gpsimd.tensor_scalar_mul(
                    wtri[:], t_triub, eR[:, ii:ii + 1])
                mp = ps_s.tile([128, C], f32, tag="sA", name=f"mp_{ii}")
                nc.tensor.matmul(mp[:], wtri[:], t_xT[:, ii, :],
                                 start=True, stop=(ii == 0))
                ap = ps_s1.tile([128, C], f32, tag="sB", name=f"ap_{ii}")
                nc.tensor.matmul(ap[:], wtri[:], x2[:, ii, :],
                                 start=True, stop=(ii == 0))
                return mp, ap

            def p2_csel(ii, mp, ap):
                if ii > 0:
                    nc.tensor.matmul(mp[:], csel_ap(ii), trowE1,
                                     start=False, stop=True)
                    nc.tensor.matmul(ap[:], csel_ap(ii), trowE2,
                                     start=False, stop=True)

            def p2_evict(ii, mp, ap):
                zp = zpp.tile([128, C], bf16, tag="zp", name=f"zp_{ii}")
                if ii % 2 == 0:
                    nc.scalar.activation(zp[:], mp[:], AF.Square,
                                         scale=R_col[:, ii:ii + 1])
                else:
                    nc.vector.scalar_tensor_tensor(zp[:], mp[:],
                                                   R2_col[:, ii:ii + 1],
                                                   mp[:],
                                                   op0=OP.mult, op1=OP.mult)
                var2 = v2b.tile([128, C], bf16, tag="v2", name=f"var2_{ii}")
                nc.vector.scalar_tensor_tensor(var2[:], ap[:],
                                               R_col[:, ii:ii + 1], zp[:],
                                               op0=OP.mult, op1=OP.subtract)
                nc.gpsimd.tensor_scalar_max(var2[:], var2[:], EPSC)
                nc.scalar.activation(wsall[:, ii, :], var2[:], AF.Sqrt)

            for g in range(4):
                i0 = 4 * g
                h = hh.tile([A, 512], bf16, tag="h_sb", name=f"h_{g}")
                nc.scalar.activation(h[:], hpres[g][:], AF.Tanh)
                ps_sc = ps_ca.tile([128, 4], f32, tag="cA",
                                   name=f"ps_sc_{g}")
                for k in range(4):
                    nc.tensor.matmul(
                        ps_sc[:, k:k + 1],
                        h[:, 128 * k:128 * (k + 1)],
                        t_w2b, start=True, stop=True)
                nc.vector.tensor_add(eTf[:, i0:i0 + 4],
                                     ps_sc[:],
                                     t_maskexp[:, i0:i0 + 4])
                nc.scalar.activation(
                    eTb[:, i0:i0 + 4], eTf[:, i0:i0 + 4], AF.Exp)
                ps_et = ps_ca.tile([16, 1], f32, tag="cA",
                                   name=f"ps_et_{g}")
                nc.tensor.matmul(ps_et[:], eTb[:], t_onescb,
                                 start=True, stop=True)
                nc.vector.tensor_copy(etot[:], ps_et[:])
                nc.tensor.matmul(eEp[:, i0:i0 + 4], t_triub,
                                 eTb[:, i0:i0 + 4],
                                 start=True, stop=False)
                nc.tensor.matmul(eEp[:, i0:i0 + 4], bcast16(etot),
                                 t_sutri16[:, i0:i0 + 4],
                                 start=False, stop=True)
                nc.vector.reciprocal(R_col[:, i0:i0 + 4],
                                     eEp[:, i0:i0 + 4])
                nc.vector.tensor_mul(R2_col[:, i0:i0 + 4],
                                     R_col[:, i0:i0 + 4],
                                     R_col[:, i0:i0 + 4])
                nc.vector.tensor_copy(eR[:, i0:i0 + 4], eTb[:, i0:i0 + 4])

                # chunk 4g: its carries come from prior groups only
                mp0, ap0 = p2_mains(i0)
                p2_csel(i0, mp0, ap0)
                # group-g e-weighted totals (both quantities in one tile)
                totE_g = ps_ca.tile([128, 8, 4], f32, tag="cA",
                                    name=f"totE_{g}")
                for kk in range(4):
                    ii = i0 + kk
                    for k in range(4):
                        nc.tensor.matmul(totE_g[:, k, kk:kk + 1],
                                         t_xT[:, ii, 128 * k:128 * (k + 1)],
                                         eTb[:, ii:ii + 1],
                                         start=True, stop=True)
                        nc.tensor.matmul(totE_g[:, 4 + k, kk:kk + 1],
                                         x2[:, ii, 128 * k:128 * (k + 1)],
                                         eTb[:, ii:ii + 1],
                                         start=True, stop=True)
                totEb_g = colp.tile([128, 8, 4], bf16, name=f"totEb_{g}",
                                    tag="totEb", bufs=2)
                nc.scalar.activation(totEb_g[:], totE_g[:], AF.Copy)
                mp1, ap1 = p2_mains(i0 + 1)
                trow_ps = ps_ca.tile([4, 8, 128], bf16, tag="cA",
                                     name=f"trow_ps_{g}")
                for k in range(8):
                    nc.tensor.matmul(trow_ps[:, k, :], totEb_g[:, k, :],
                                     t_identb, is_transpose=True,
                                     start=True, stop=True)
                nc.scalar.activation(trowE[i0:i0 + 4, :, :],
                                     trow_ps[:], AF.Copy)
                p2_evict(i0, mp0, ap0)
                p2_csel(i0 + 1, mp1, ap1)
                p2_evict(i0 + 1, mp1, ap1)
                for kk in (2, 3):
                    ii = i0 + kk
                    mp, ap = p2_mains(ii)
                    p2_csel(ii, mp, ap)
                    p2_evict(ii, mp, ap)
                if g == 3:
                    # G machinery + final-mean as soon as R is complete
                    RLr = colp.tile([128, NCH], f32r)
                    nc.vector.tensor_mul(RLr[:], R_col[:],
                                         t_finalw.bitcast(f32))
                    ps_G = ps_cb.tile([128, NCH], f32, tag="cB")
                    nc.tensor.matmul(ps_G[:], t_tril, RLr[:],
                                     start=True, stop=False)
                    ps_rt = ps_ca.tile([16, 1], f32, tag="cA")
                    nc.tensor.matmul(ps_rt[:], RLr[:].bitcast(f32),
                                     t_onescf, start=True, stop=True)
                    rtot = colp.tile([16, 1], f32)
                    nc.vector.tensor_copy(rtot[:], ps_rt[:])
                    nc.tensor.matmul(ps_G[:], bcast16(rtot), t_ltri16,
                                     start=False, stop=True)
                    wG = colp.tile([128, NCH], bf16)
                    nc.vector.tensor_mul(wG[:], eTb[:], ps_G[:])
                    for k in range(4):
                        for ii in range(NCH):
                            nc.tensor.matmul(
                                outaccM[:, k:k + 1],
                                t_xT[:, ii, 128 * k:128 * (k + 1)],
                                wG[:, ii:ii + 1],
                                start=(ii == 0), stop=(ii == NCH - 1))
                    outM = colp.tile([128, 4], f32)
                    nc.scalar.activation(outM[:], outaccM[:], AF.Copy)
                    nc.sync.dma_start(d_out[:, 0:4], outM[:])

            # final-std reductions: one PSUM column at a time
            for k in range(4):
                for ii in range(NCH):
                    nc.tensor.matmul(outaccS[:, k:k + 1],
                                     wsall[:, ii, 128 * k:128 * (k + 1)],
                                     t_fwb[:, ii:ii + 1],
                                     start=(ii == 0),
                                     stop=(ii == NCH - 1))
            outS = colp.tile([128, 4], f32)
            nc.scalar.activation(outS[:], outaccS[:], AF.Copy)
            nc.sync.dma_start(d_out[:, 4:8], outS[:])

    nc.insert_act_table_loads = lambda: None
    nc.compile()
    return nc


def _host_inputs(x, lengths, w1, b1, w2, b2):
    x = np.asarray(x, np.float32)
    lengths = np.asarray(lengths)
    w1 = np.asarray(w1, np.float32)
    b1 = np.asarray(b1, np.float32)
    w2 = np.asarray(w2, np.float32)
    b2 = np.asarray(b2, np.float32)

    sutri16 = np.triu(np.ones((16, 16), np.float32), 1)

    cstf0 = np.zeros((128, NF), np.float32)
    cstf0[:, CF_B1] = b1
    cstf0[:, CF_EPS] = EPSC
    cstf0[0:16, CF_SUTRI16:CF_SUTRI16 + 16] = sutri16
    cstf0[0:16, CF_LTRI16:CF_LTRI16 + 16] = np.tril(
        np.ones((16, 16), np.float32), -1)
    cstf0[:, CF_ONESC] = 1.0
    cstf0[:, CF_IDENT:CF_IDENT + 128] = np.eye(128, dtype=np.float32)

    cstr = np.zeros((128, NR), np.float32)
    cstr[:, CR_TRIL:CR_TRIL + 128] = np.tril(np.ones((128, 128), np.float32))
    cstr[:, CR_ONESC] = 1.0

    cstb0 = np.zeros((128, NB), np.float32)
    cstb0[:, CB_TRIU:CB_TRIU + 128] = np.triu(np.ones((128, 128), np.float32))
    cstb0[0:16, CB_SUTRI16B:CB_SUTRI16B + 16] = sutri16
    cstb0[:, CB_W2] = w2[:, 0]
    cstb0[:, CB_ONESC] = 1.0
    cstb0[:, CB_IDENTB:CB_IDENTB + 128] = np.eye(128, dtype=np.float32)

    tt = np.arange(T)
    F8 = ml_dtypes.float8_e4m3
    w1r = np.ascontiguousarray(
        w1.reshape(12, 128, A).transpose(1, 0, 2).reshape(128, 12 * A))
    w1x8 = w1r[:, :8 * A].astype(F8)
    w1sb = w1r[:, 8 * A:].astype(BF)

    maps = []
    for b in range(B):
        L = int(lengths[b])
        rcnt = (1.0 / np.minimum(tt + 1, max(L, 1))).astype(np.float32)
        maskexp = (float(b2[0]) +
                   np.where(tt < L, 0.0, NEG)).astype(np.float32)
        finalw = np.where(tt < L, 1.0 / max(L, 1), 0.0).astype(np.float32)
        cstf = cstf0.copy()
        cstf[:, CF_RCNT:CF_RCNT + 16] = rcnt.reshape(NCH, 128).T
        cstf[:, CF_RCNT2:CF_RCNT2 + 16] = (rcnt * rcnt).reshape(NCH, 128).T
        cstf[:, CF_RCNT2:CF_RCNT2 + 16] = (rcnt * rcnt).reshape(NCH, 128).T
        cstf[:, CF_MASKEXP:CF_MASKEXP + 16] = maskexp.reshape(NCH, 128).T
        cstr_b = cstr.copy()
        cstr_b[:, CR_FINALW:CR_FINALW + 16] = finalw.reshape(NCH, 128).T
        cstb_b = cstb.copy()
        cstb_b[:, CB_FWB:CB_FWB + 16] = finalw.reshape(NCH, 128).T
        cstb = cstb0.copy()
        cstb[:, CB_FWB:CB_FWB + 16] = finalw.reshape(NCH, 128).T
        maps.append({
            "xT": np.ascontiguousarray(x[b].T).astype(BF),
            "xN": np.ascontiguousarray(x[b]).astype(F8),
            "w1x": w1x8,
            "w1s": w1sb,
            "cstf": cstf,
            "cstr": cstr_b,
            "cstb": cstb.astype(BF),
            "rcntb": np.ascontiguousarray(
                np.broadcast_to(rcnt[None, :], (128, T))).astype(BF),
        })
    return maps


def kernel(x, lengths, w1, b1, w2, b2):
    from concourse.bass_utils import run_bass_kernel_spmd

    if "nc" not in _CACHE:
        _CACHE["nc"] = _build()
    nc = _CACHE["nc"]
    maps = _host_inputs(x, lengths, w1, b1, w2, b2)
    res = run_bass_kernel_spmd(nc, maps, list(range(B))).results
    rows = []
    for b in range(B):
        arr = np.asarray(res[b]["out"], np.float32)  # (128, 8)
        rows.append(np.concatenate([arr[:, 0:4].T.reshape(-1),
                                    arr[:, 4:8].T.reshape(-1)]))
    return np.stack(rows, axis=0).astype(np.float32)
